# revision 16
# baseline (speedup 1.0000x reference)
"""GQA kernel for Trainium2, 8 NeuronCores — query-sharded, collective-free.

Problem: B=2, T=2048, HIDDEN=1024, 16 q-heads, 4 kv-heads, head_dim=64,
causal attention + output projection.

Sharding: core = (batch b = core//4, q-interleave g = core%4). Each core
handles ALL 16 heads for q-tiles {4c+g : c=0..3} (4 x 128 queries). KV is
computed redundantly on every core of a batch group, so qkv, attention AND
o_proj are fully local: no collectives at all (collective latency in this
environment is large and wildly variable).

Causal balance: chunk-slot c processes q-tile 4c+g against key tiles
0..4c+3 (uniform program). Key tiles beyond the causal limit (jd > g) are
killed via a per-partition bias of -30 fed to the EXP activation
(exp(s-30) ~ 0); the diagonal tile (jd == g) gets a triangular 0/1
mask-multiply (per-core mask data selects triangle vs all-ones so the
instruction stream stays uniform across cores).

Device dataflow (all matmuls bf16, fp32 PSUM):
  - q-proj over xq (own queries, compact): head-pair M-blocks -> qT
    [d, head, 512q], d duplicated on both partition halves for row-packing
  - kT via W-stationary matmuls ([g0|g1] and [g2|g3] partition-stacked,
    which row-packs directly into the score matmuls)
  - V natural directly: x_tile.T @ Wv -> [128 keys, 256 vdims]; ones col
    appended -> softmax denominators fall out of the PV matmul
  - scores: per (j, group-half): 2 row-packed matmuls K=64 -> s2 [128k,
    2 groups x 4 heads x 128q]; EXP on ACT with causal bias; PV per group
  - normalize: denominator rows collected at 32-aligned partitions, ONE
    DVE reciprocal per chunk, gpsimd broadcast, per-head muls into attnT
  - o_proj local: Wo.T @ attnT per slot; slots 0-1 batched (N=256) after
    chunk-2 attention, slot 2 after chunk-3, slot 3 is the only tail.
  - output outT [1024, 512 own q] f32; host re-interleaves.
"""

import sys

import numpy as np

try:
    import concourse.bass as bass
except ImportError:
    sys.path.insert(0, "/opt/trn_rl_repo")
    import concourse.bass as bass

import ml_dtypes
from contextlib import ExitStack

import concourse.tile as tile
from concourse import bacc, mybir
from concourse.bass import ds, ts
from concourse.bass_utils import run_bass_kernel_spmd

BF16 = mybir.dt.bfloat16
F32 = mybir.dt.float32

P = 128
T = 2048
HID = 1024
KT = HID // P   # 8 k-tiles over hidden
CH = 512        # q columns per core (4 tiles of 128)
D = 64
SCALE = D ** -0.5
NEG = -30.0

_PROGRAM = None


def build_program():
    nc = bacc.Bacc(num_devices=8)

    xT_d = nc.declare_dram_parameter("xT", [HID, T], BF16, isOutput=False)
    xq_d = nc.declare_dram_parameter("xq", [HID, CH], BF16, isOutput=False)
    wq_d = nc.declare_dram_parameter("wq", [HID, 1024], BF16, isOutput=False)
    wkv_d = nc.declare_dram_parameter("wkv", [HID, 512], BF16, isOutput=False)
    wo_d = nc.declare_dram_parameter("wo", [HID, 1024], BF16, isOutput=False)
    mask_d = nc.declare_dram_parameter("maskq", [P, 4 * 1024], BF16, isOutput=False)
    bias_d = nc.declare_dram_parameter("biasb", [P, 4], F32, isOutput=False)
    outT_d = nc.declare_dram_parameter("outT", [HID, CH], BF16, isOutput=True)

    with tile.TileContext(nc) as tc, ExitStack() as ctx:
        sing = ctx.enter_context(tc.tile_pool(name="sing", bufs=1))
        work = ctx.enter_context(tc.tile_pool(name="work", bufs=2, space="PSUM"))
        accp = ctx.enter_context(tc.tile_pool(name="accp", bufs=4, space="PSUM"))
        ptp = ctx.enter_context(tc.tile_pool(name="ptp", bufs=6))
        outp = ctx.enter_context(tc.tile_pool(name="outp", bufs=3))
        nrmp = ctx.enter_context(tc.tile_pool(name="nrmp", bufs=2))
        oevp = ctx.enter_context(tc.tile_pool(name="oevp", bufs=6))

        # --- loads needed before chunk-0 compute ---
        xq_sb = sing.tile([P, KT, CH], BF16)
        nc.sync.dma_start(xq_sb, xq_d[:, :].rearrange("(kt p) n -> p kt n", p=P))
        wq_sb = sing.tile([P, KT, 1024], BF16)
        nc.sync.dma_start(
            wq_sb[:, :, 0:512],
            wq_d[:, 0:512].rearrange("(kt p) n -> p kt n", p=P),
        )
        wkv_sb = sing.tile([P, KT, 512], BF16)
        nc.sync.dma_start(wkv_sb, wkv_d[:, :].rearrange("(kt p) n -> p kt n", p=P))
        xT_sb = sing.tile([P, KT, T], BF16)
        for kt in range(KT):
            nc.sync.dma_start(xT_sb[:, kt, ts(0, CH)], xT_d[ts(kt, P), ts(0, CH)])
        nc.sync.dma_start(
            wq_sb[:, :, 512:1024],
            wq_d[:, 512:1024].rearrange("(kt p) n -> p kt n", p=P),
        )
        maskq = sing.tile([P, 4, 1024], BF16)
        nc.sync.dma_start(
            maskq, mask_d[:, :].rearrange("p (v n) -> p v n", v=4)
        )
        biasb = sing.tile([P, 4], F32)
        nc.sync.dma_start(biasb, bias_d[:, :])
        # --- deferred loads ---
        for c in range(1, 4):
            for kt in range(KT):
                nc.sync.dma_start(
                    xT_sb[:, kt, ts(c, CH)], xT_d[ts(kt, P), ts(c, CH)]
                )
        wo_sb = sing.tile([P, KT, 1024], BF16)
        nc.sync.dma_start(wo_sb, wo_d[:, :].rearrange("(kt p) n -> p kt n", p=P))

        actwarm = sing.tile([P, 1], F32)
        nc.gpsimd.memset(actwarm, 0.0)
        nc.scalar.activation(actwarm, actwarm, mybir.ActivationFunctionType.Exp)

        qT_sb = sing.tile([P, 16, CH], BF16)      # [d dup'd on both halves, head, own q]
        kT_sb = sing.tile([P, 2, T], BF16)        # [g-pair dims, block, keys]
        vaug = sing.tile([P, 16, 4, 66], BF16)    # [keys, tile, group, 64+1]
        nc.gpsimd.memset(vaug[:, :, :, 64:65], 1.0)
        attnT = sing.tile([P, KT, CH], BF16)      # normalized attn^T for o_proj

        # ---- q-proj: all 4 slots ----
        def emit_qproj(mhs):
            for mh in mhs:
                qp = work.tile([P, CH], F32, tag="work", name=f"qp{mh}")
                for kt in range(KT):
                    nc.tensor.matmul(
                        qp,
                        wq_sb[:, kt, ts(mh, P)],
                        xq_sb[:, kt, :],
                        start=(kt == 0),
                        stop=(kt == KT - 1),
                    )
                nc.vector.tensor_copy(qT_sb[0:64, 2 * mh, :], qp[0:64, :])
                nc.vector.tensor_copy(qT_sb[64:128, 2 * mh + 1, :], qp[64:128, :])
                nc.sync.dma_start(
                    qT_sb[64:128, 2 * mh, :], qT_sb[0:64, 2 * mh, :]
                )
                nc.sync.dma_start(
                    qT_sb[0:64, 2 * mh + 1, :], qT_sb[64:128, 2 * mh + 1, :]
                )

        def emit_oproj(c0, nsl, mbs=range(8)):
            # local o_proj for slots [c0, c0+nsl), output blocks mbs
            w = 128 * nsl
            for mb in mbs:
                op = work.tile([P, CH], F32, tag="work", name=f"op{c0}_{mb}")
                for kt in range(KT):
                    nc.tensor.matmul(
                        op[:, 0:w],
                        wo_sb[:, kt, ts(mb, P)],
                        attnT[:, kt, ds(128 * c0, w)],
                        start=(kt == 0),
                        stop=(kt == KT - 1),
                    )
                ob = outp.tile([P, w], BF16, tag="ob", name=f"ob{c0}_{mb}")
                nc.vector.tensor_copy(ob, op[:, 0:w])
                nc.sync.dma_start(outT_d[ts(mb, P), ds(128 * c0, w)], ob)

        def emit_kv(c):
            # kv-proj for key chunk c (keys 512c .. 512c+511)
            for mb in range(2):
                kp = work.tile([P, CH], F32, tag="work", name=f"kp{c}_{mb}")
                for kt in range(KT):
                    nc.tensor.matmul(
                        kp,
                        wkv_sb[:, kt, ts(mb, P)],
                        xT_sb[:, kt, ts(c, CH)],
                        start=(kt == 0),
                        stop=(kt == KT - 1),
                    )
                nc.vector.tensor_copy(kT_sb[:, mb, ts(c, CH)], kp)
            for jt in range(4):
                j = 4 * c + jt
                vp = work.tile([P, 256], F32, tag="work", name=f"vp{j}")
                for kt in range(KT):
                    nc.tensor.matmul(
                        vp,
                        xT_sb[:, kt, ts(j, P)],
                        wkv_sb[:, kt, ds(256, 256)],
                        start=(kt == 0),
                        stop=(kt == KT - 1),
                    )
                nc.vector.tensor_copy(vaug[:, j, :, 0:64], vp)

        emit_qproj(range(8))

        # Slot order 0,1,3,2: the LAST chunk processed is the short one
        # (12 key tiles), shrinking the serial tail (norm + o_proj). kv
        # chunks 2 and 3 are produced inside slot 3's long early j-loop.
        # Old key tiles (j < 4c) don't need this chunk's kv, so each
        # j-loop starts immediately at the chunk boundary and the kv-proj
        # matmuls hide under the ACT-bound early iterations.
        KV_AT = {0: {0: 0}, 1: {4: 1}, 2: {8: 2}, 3: {12: 3}}
        for c in (0, 1, 2, 3):
            # ---- attention for slot c (q-tile 4c+g, 128 queries) ----
            ntk = 4 * (c + 1)
            oa = [
                accp.tile([P, CH], F32, tag="acc", name=f"oa{c}_{gr}")
                for gr in range(4)
            ]
            for j in range(ntk):
                jd = j - 4 * c  # >= 0 -> diagonal region
                if j in KV_AT[c]:
                    emit_kv(KV_AT[c][j])
                for gh in range(2):
                    s2 = work.tile(
                        [P, 1024], F32, tag="work", name=f"s2_{c}_{j}_{gh}"
                    )
                    nc.tensor.matmul(
                        s2[:, 0:CH],
                        kT_sb[0:64, gh, ts(j, P)],
                        qT_sb[0:64, ds(8 * gh, 4), ds(128 * c, P)],
                        start=True,
                        stop=True,
                        tile_position=(0, 0),
                    )
                    nc.tensor.matmul(
                        s2[:, CH:1024],
                        kT_sb[64:128, gh, ts(j, P)],
                        qT_sb[64:128, ds(8 * gh + 4, 4), ds(128 * c, P)],
                        start=True,
                        stop=True,
                        tile_position=(64, 0),
                    )
                    pt = ptp.tile([P, 1024], BF16, tag="pt", name=f"pt{c}_{j}_{gh}")
                    if jd >= 0:
                        # bias kills fully-masked tiles (jd > g): exp(s-30)~0
                        nc.scalar.activation(
                            pt, s2, mybir.ActivationFunctionType.Exp,
                            bias=biasb[:, jd : jd + 1],
                        )
                        # triangle on the diagonal tile (mask data is
                        # all-ones on cores where jd != g)
                        nc.vector.tensor_mul(pt, pt, maskq[:, jd, :])
                    else:
                        nc.scalar.activation(
                            pt, s2, mybir.ActivationFunctionType.Exp
                        )
                    for gg in range(2):
                        gr = 2 * gh + gg
                        nc.tensor.matmul(
                            oa[gr][0:65, :],
                            vaug[:, j, gr, 0:65],
                            pt[:, ts(gg, CH)],
                            start=(j == 0),
                            stop=(j == ntk - 1),
                        )

            # o_proj for ALREADY-normalized earlier slots goes here, right
            # after this chunk's attention matmuls: it overlaps this chunk's
            # trailing evac/normalize (DVE) instead of sitting in the tail.
            if c == 2:
                emit_oproj(0, 2, range(4))
            elif c == 3:
                # the rest of slots 0-1 plus slot 2: this PE work fills the
                # tail window while chunk 3's normalize runs on DVE/gpsimd
                emit_oproj(0, 2, range(4, 8))
                emit_oproj(2, 1)

            # ---- evacuate + normalize into attnT ----
            # tail chunk: den rows + reciprocal FIRST (shortens the
            # norm->o_proj critical chain); other chunks: oev first so the
            # oa PSUM banks free up for the next chunk's accumulators
            den128 = nrmp.tile([P, CH], F32, tag="den", name=f"den{c}")
            rcp128 = nrmp.tile([P, CH], F32, tag="rcp", name=f"rcp{c}")
            oev = []
            if c == 3:
                for gr in range(4):
                    nc.vector.tensor_copy(
                        den128[ds(32 * gr, 1), :], oa[gr][64:65, :]
                    )
                nc.vector.reciprocal(rcp128, den128)
                for gr in range(4):
                    oe = oevp.tile([64, CH], F32, tag="oev", name=f"oev{c}_{gr}")
                    nc.vector.tensor_copy(oe, oa[gr][0:64, :])
                    oev.append(oe)
            else:
                for gr in range(4):
                    oe = oevp.tile([64, CH], F32, tag="oev", name=f"oev{c}_{gr}")
                    nc.vector.tensor_copy(oe, oa[gr][0:64, :])
                    nc.vector.tensor_copy(
                        den128[ds(32 * gr, 1), :], oa[gr][64:65, :]
                    )
                    oev.append(oe)
                nc.vector.reciprocal(rcp128, den128)
            for gr in range(4):
                rch = nrmp.tile([1, CH], F32, tag="rch", name=f"rch{c}_{gr}")
                nc.vector.tensor_copy(rch, rcp128[ds(32 * gr, 1), :])
                rb = nrmp.tile([64, CH], F32, tag="rb", name=f"rb{c}_{gr}")
                nc.gpsimd.partition_broadcast(rb, rch)
                for hh in range(4):
                    h = 4 * gr + hh
                    nc.vector.tensor_mul(
                        attnT[ds(64 * (h % 2), 64), h // 2, ds(128 * c, P)],
                        oev[gr][:, ts(hh, P)],
                        rb[:, ts(hh, P)],
                    )

        emit_oproj(3, 1)

    nc.finalize()
    return nc


def _prep_inputs(x, Wq, Wkv, Wo):
    bf = ml_dtypes.bfloat16
    x = np.asarray(x, dtype=np.float32)
    Wq = np.asarray(Wq, dtype=np.float32)
    Wkv = np.asarray(Wkv, dtype=np.float32)
    Wo = np.asarray(Wo, dtype=np.float32)

    xT = [np.ascontiguousarray(x[b].T).astype(bf) for b in range(2)]
    wq = np.ascontiguousarray(Wq * SCALE).astype(bf)
    wkv = np.ascontiguousarray(Wkv).astype(bf)
    wo = np.ascontiguousarray(Wo).astype(bf)

    # triangular within-tile causal mask, replicated across the 8 head slots
    kk = np.arange(P)
    tri = (kk[:, None] <= kk[None, :]).astype(bf)          # [128 k, 128 q]
    tri8 = np.tile(tri, (1, 8))                            # [128, 1024]
    ones8 = np.ones((P, 1024), dtype=bf)

    in_maps = []
    for core in range(8):
        b, g = core // 4, core % 4
        qtiles = [4 * c + g for c in range(4)]
        xq = np.ascontiguousarray(
            np.concatenate([x[b, 128 * t : 128 * t + 128, :] for t in qtiles]).T
        ).astype(bf)
        maskq = np.concatenate(
            [tri8 if jd == g else ones8 for jd in range(4)], axis=1
        )  # [128, 4*1024]
        biasb = np.zeros((P, 4), dtype=np.float32)
        for jd in range(4):
            if jd > g:
                biasb[:, jd] = NEG
        in_maps.append(
            {
                "xT": xT[b],
                "xq": xq,
                "wq": wq,
                "wkv": wkv,
                "wo": wo,
                "maskq": np.ascontiguousarray(maskq),
                "biasb": biasb,
            }
        )
    return in_maps


def run(x, Wq, Wkv, Wo, trace=False, **trace_kwargs):
    global _PROGRAM
    if _PROGRAM is None:
        _PROGRAM = build_program()
    nc = _PROGRAM
    in_maps = _prep_inputs(x, Wq, Wkv, Wo)
    res = run_bass_kernel_spmd(
        nc, in_maps, core_ids=list(range(8)), trace=trace, **trace_kwargs
    )
    outs = res.results
    full = np.empty((2, T, HID), dtype=np.float32)
    for core in range(8):
        b, g = core // 4, core % 4
        outT = np.asarray(outs[core]["outT"]).astype(np.float32)
        for c in range(4):
            t = 4 * c + g
            full[b, 128 * t : 128 * t + 128, :] = outT[:, 128 * c : 128 * c + 128].T
    return full, res


def kernel(x, Wq, Wkv, Wo):
    out, _ = run(x, Wq, Wkv, Wo, trace=False)
    return out


# revision 17
# speedup vs baseline: 1.1447x; 1.1447x over previous
"""GQA kernel for Trainium2, 8 NeuronCores — query-sharded, collective-free.

Problem: B=2, T=2048, HIDDEN=1024, 16 q-heads, 4 kv-heads, head_dim=64,
causal attention + output projection.

Sharding: core = (batch b = core//4, q-interleave g = core%4). Each core
handles ALL 16 heads for q-tiles {4c+g : c=0..3} (4 x 128 queries). KV is
computed redundantly on every core of a batch group, so qkv, attention AND
o_proj are fully local: no collectives at all (collective latency in this
environment is large and wildly variable).

Causal balance: chunk-slot c processes q-tile 4c+g against key tiles
0..4c+3 (uniform program). Key tiles beyond the causal limit (jd > g) are
killed via a per-partition bias of -30 fed to the EXP activation
(exp(s-30) ~ 0); the diagonal tile (jd == g) gets a triangular 0/1
mask-multiply (per-core mask data selects triangle vs all-ones so the
instruction stream stays uniform across cores).

Device dataflow (all matmuls bf16, fp32 PSUM):
  - q-proj over xq (own queries, compact): head-pair M-blocks -> qT
    [d, head, 512q], d duplicated on both partition halves for row-packing
  - kT via W-stationary matmuls ([g0|g1] and [g2|g3] partition-stacked,
    which row-packs directly into the score matmuls)
  - V natural directly: x_tile.T @ Wv -> [128 keys, 256 vdims]; ones col
    appended -> softmax denominators fall out of the PV matmul
  - scores: per (j, group-half): 2 row-packed matmuls K=64 -> s2 [128k,
    2 groups x 4 heads x 128q]; EXP on ACT with causal bias; PV per group
  - normalize: denominator rows collected at 32-aligned partitions, ONE
    DVE reciprocal per chunk, gpsimd broadcast, per-head muls into attnT
  - o_proj local: Wo.T @ attnT per slot; slots 0-1 batched (N=256) after
    chunk-2 attention, slot 2 after chunk-3, slot 3 is the only tail.
  - output outT [1024, 512 own q] f32; host re-interleaves.
"""

import sys

import numpy as np

try:
    import concourse.bass as bass
except ImportError:
    sys.path.insert(0, "/opt/trn_rl_repo")
    import concourse.bass as bass

import ml_dtypes
from contextlib import ExitStack

import concourse.tile as tile
from concourse import bacc, mybir
from concourse.bass import ds, ts
from concourse.bass_utils import run_bass_kernel_spmd

BF16 = mybir.dt.bfloat16
F32 = mybir.dt.float32

P = 128
T = 2048
HID = 1024
KT = HID // P   # 8 k-tiles over hidden
CH = 512        # q columns per core (4 tiles of 128)
D = 64
SCALE = D ** -0.5
NEG = -30.0

_PROGRAM = None


def build_program():
    nc = bacc.Bacc(num_devices=8)

    xT_d = nc.declare_dram_parameter("xT", [HID, T], BF16, isOutput=False)
    xq_d = nc.declare_dram_parameter("xq", [HID, CH], BF16, isOutput=False)
    wq_d = nc.declare_dram_parameter("wq", [HID, 1024], BF16, isOutput=False)
    wkv_d = nc.declare_dram_parameter("wkv", [HID, 512], BF16, isOutput=False)
    wo_d = nc.declare_dram_parameter("wo", [HID, 1024], BF16, isOutput=False)
    mask_d = nc.declare_dram_parameter("maskq", [P, 4 * 1024], BF16, isOutput=False)
    bias_d = nc.declare_dram_parameter("biasb", [P, 4], F32, isOutput=False)
    outT_d = nc.declare_dram_parameter("outT", [HID, CH], BF16, isOutput=True)

    with tile.TileContext(nc) as tc, ExitStack() as ctx:
        sing = ctx.enter_context(tc.tile_pool(name="sing", bufs=1))
        work = ctx.enter_context(tc.tile_pool(name="work", bufs=2, space="PSUM"))
        accp = ctx.enter_context(tc.tile_pool(name="accp", bufs=4, space="PSUM"))
        ptp = ctx.enter_context(tc.tile_pool(name="ptp", bufs=6))
        outp = ctx.enter_context(tc.tile_pool(name="outp", bufs=3))
        nrmp = ctx.enter_context(tc.tile_pool(name="nrmp", bufs=2))
        oevp = ctx.enter_context(tc.tile_pool(name="oevp", bufs=6))

        # --- loads needed before chunk-0 compute ---
        xq_sb = sing.tile([P, KT, CH], BF16)
        nc.sync.dma_start(xq_sb, xq_d[:, :].rearrange("(kt p) n -> p kt n", p=P))
        wq_sb = sing.tile([P, KT, 1024], BF16)
        nc.sync.dma_start(
            wq_sb[:, :, 0:512],
            wq_d[:, 0:512].rearrange("(kt p) n -> p kt n", p=P),
        )
        wkv_sb = sing.tile([P, KT, 512], BF16)
        nc.sync.dma_start(wkv_sb, wkv_d[:, :].rearrange("(kt p) n -> p kt n", p=P))
        xT_sb = sing.tile([P, KT, T], BF16)
        for kt in range(KT):
            nc.sync.dma_start(xT_sb[:, kt, ts(0, CH)], xT_d[ts(kt, P), ts(0, CH)])
        nc.sync.dma_start(
            wq_sb[:, :, 512:1024],
            wq_d[:, 512:1024].rearrange("(kt p) n -> p kt n", p=P),
        )
        maskq = sing.tile([P, 4, 1024], BF16)
        nc.sync.dma_start(
            maskq, mask_d[:, :].rearrange("p (v n) -> p v n", v=4)
        )
        biasb = sing.tile([P, 4], F32)
        nc.sync.dma_start(biasb, bias_d[:, :])
        # --- deferred loads ---
        for c in range(1, 4):
            for kt in range(KT):
                nc.sync.dma_start(
                    xT_sb[:, kt, ts(c, CH)], xT_d[ts(kt, P), ts(c, CH)]
                )
        wo_sb = sing.tile([P, KT, 1024], BF16)
        nc.sync.dma_start(wo_sb, wo_d[:, :].rearrange("(kt p) n -> p kt n", p=P))

        actwarm = sing.tile([P, 1], F32)
        nc.gpsimd.memset(actwarm, 0.0)
        nc.scalar.activation(actwarm, actwarm, mybir.ActivationFunctionType.Exp)

        qT_sb = sing.tile([P, 16, CH], BF16)      # [d dup'd on both halves, head, own q]
        kT_sb = sing.tile([P, 2, T], BF16)        # [g-pair dims, block, keys]
        vaug = sing.tile([P, 16, 4, 66], BF16)    # [keys, tile, group, 64+1]
        nc.gpsimd.memset(vaug[:, :, :, 64:65], 1.0)
        attnT = sing.tile([P, KT, CH], BF16)      # normalized attn^T for o_proj

        # ---- q-proj: all 4 slots ----
        def emit_qproj(mhs):
            for mh in mhs:
                qp = work.tile([P, CH], F32, tag="work", name=f"qp{mh}")
                for kt in range(KT):
                    nc.tensor.matmul(
                        qp,
                        wq_sb[:, kt, ts(mh, P)],
                        xq_sb[:, kt, :],
                        start=(kt == 0),
                        stop=(kt == KT - 1),
                    )
                nc.vector.tensor_copy(qT_sb[0:64, 2 * mh, :], qp[0:64, :])
                nc.vector.tensor_copy(qT_sb[64:128, 2 * mh + 1, :], qp[64:128, :])
                nc.sync.dma_start(
                    qT_sb[64:128, 2 * mh, :], qT_sb[0:64, 2 * mh, :]
                )
                nc.sync.dma_start(
                    qT_sb[0:64, 2 * mh + 1, :], qT_sb[64:128, 2 * mh + 1, :]
                )

        def emit_oproj(c0, nsl, mbs=range(8)):
            # local o_proj for slots [c0, c0+nsl), output blocks mbs
            w = 128 * nsl
            for mb in mbs:
                op = work.tile([P, CH], F32, tag="work", name=f"op{c0}_{mb}")
                for kt in range(KT):
                    nc.tensor.matmul(
                        op[:, 0:w],
                        wo_sb[:, kt, ts(mb, P)],
                        attnT[:, kt, ds(128 * c0, w)],
                        start=(kt == 0),
                        stop=(kt == KT - 1),
                    )
                ob = outp.tile([P, w], BF16, tag="ob", name=f"ob{c0}_{mb}")
                nc.vector.tensor_copy(ob, op[:, 0:w])
                nc.sync.dma_start(outT_d[ts(mb, P), ds(128 * c0, w)], ob)

        def emit_kv(c):
            # kv-proj for key chunk c (keys 512c .. 512c+511)
            for mb in range(2):
                kp = work.tile([P, CH], F32, tag="work", name=f"kp{c}_{mb}")
                for kt in range(KT):
                    nc.tensor.matmul(
                        kp,
                        wkv_sb[:, kt, ts(mb, P)],
                        xT_sb[:, kt, ts(c, CH)],
                        start=(kt == 0),
                        stop=(kt == KT - 1),
                    )
                nc.vector.tensor_copy(kT_sb[:, mb, ts(c, CH)], kp)
            for jt in range(4):
                j = 4 * c + jt
                vp = work.tile([P, 256], F32, tag="work", name=f"vp{j}")
                for kt in range(KT):
                    nc.tensor.matmul(
                        vp,
                        xT_sb[:, kt, ts(j, P)],
                        wkv_sb[:, kt, ds(256, 256)],
                        start=(kt == 0),
                        stop=(kt == KT - 1),
                    )
                nc.vector.tensor_copy(vaug[:, j, :, 0:64], vp)

        emit_qproj(range(8))

        # Slot order 0,1,3,2: the LAST chunk processed is the short one
        # (12 key tiles), shrinking the serial tail (norm + o_proj). kv
        # chunks 2 and 3 are produced inside slot 3's long early j-loop.
        # Old key tiles (j < 4c) don't need this chunk's kv, so each
        # j-loop starts immediately at the chunk boundary and the kv-proj
        # matmuls hide under the ACT-bound early iterations.
        KV_AT = {0: {0: 0}, 1: {4: 1}, 2: {8: 2}, 3: {12: 3}}
        for c in (0, 1, 2, 3):
            # ---- attention for slot c (q-tile 4c+g, 128 queries) ----
            ntk = 4 * (c + 1)
            oa = [
                accp.tile([P, CH], F32, tag="acc", name=f"oa{c}_{gr}")
                for gr in range(4)
            ]
            for j in range(ntk):
                jd = j - 4 * c  # >= 0 -> diagonal region
                if j in KV_AT[c]:
                    emit_kv(KV_AT[c][j])
                for gh in range(2):
                    s2 = work.tile(
                        [P, 1024], F32, tag="work", name=f"s2_{c}_{j}_{gh}"
                    )
                    nc.tensor.matmul(
                        s2[:, 0:CH],
                        kT_sb[0:64, gh, ts(j, P)],
                        qT_sb[0:64, ds(8 * gh, 4), ds(128 * c, P)],
                        start=True,
                        stop=True,
                        tile_position=(0, 0),
                    )
                    nc.tensor.matmul(
                        s2[:, CH:1024],
                        kT_sb[64:128, gh, ts(j, P)],
                        qT_sb[64:128, ds(8 * gh + 4, 4), ds(128 * c, P)],
                        start=True,
                        stop=True,
                        tile_position=(64, 0),
                    )
                    pt = ptp.tile([P, 1024], BF16, tag="pt", name=f"pt{c}_{j}_{gh}")
                    if jd >= 0:
                        # bias kills fully-masked tiles (jd > g): exp(s-30)~0
                        nc.scalar.activation(
                            pt, s2, mybir.ActivationFunctionType.Exp,
                            bias=biasb[:, jd : jd + 1],
                        )
                        # triangle on the diagonal tile (mask data is
                        # all-ones on cores where jd != g)
                        nc.vector.tensor_mul(pt, pt, maskq[:, jd, :])
                    else:
                        nc.scalar.activation(
                            pt, s2, mybir.ActivationFunctionType.Exp
                        )
                    for gg in range(2):
                        gr = 2 * gh + gg
                        nc.tensor.matmul(
                            oa[gr][0:65, :],
                            vaug[:, j, gr, 0:65],
                            pt[:, ts(gg, CH)],
                            start=(j == 0),
                            stop=(j == ntk - 1),
                        )

            # o_proj for ALREADY-normalized earlier slots goes here, right
            # after this chunk's attention matmuls: it overlaps this chunk's
            # trailing evac/normalize (DVE) instead of sitting in the tail.
            if c == 2:
                emit_oproj(0, 2)
            elif c == 3:
                emit_oproj(2, 1)

            # ---- evacuate + normalize into attnT ----
            # tail chunk: den rows + reciprocal FIRST (shortens the
            # norm->o_proj critical chain); other chunks: oev first so the
            # oa PSUM banks free up for the next chunk's accumulators
            den128 = nrmp.tile([P, CH], F32, tag="den", name=f"den{c}")
            rcp128 = nrmp.tile([P, CH], F32, tag="rcp", name=f"rcp{c}")
            oev = []
            if c == 3:
                for gr in range(4):
                    nc.vector.tensor_copy(
                        den128[ds(32 * gr, 1), :], oa[gr][64:65, :]
                    )
                nc.vector.reciprocal(rcp128, den128)
                for gr in range(4):
                    oe = oevp.tile([64, CH], F32, tag="oev", name=f"oev{c}_{gr}")
                    nc.vector.tensor_copy(oe, oa[gr][0:64, :])
                    oev.append(oe)
            else:
                for gr in range(4):
                    oe = oevp.tile([64, CH], F32, tag="oev", name=f"oev{c}_{gr}")
                    nc.vector.tensor_copy(oe, oa[gr][0:64, :])
                    nc.vector.tensor_copy(
                        den128[ds(32 * gr, 1), :], oa[gr][64:65, :]
                    )
                    oev.append(oe)
                nc.vector.reciprocal(rcp128, den128)
            for gr in range(4):
                rch = nrmp.tile([1, CH], F32, tag="rch", name=f"rch{c}_{gr}")
                nc.vector.tensor_copy(rch, rcp128[ds(32 * gr, 1), :])
                rb = nrmp.tile([64, CH], F32, tag="rb", name=f"rb{c}_{gr}")
                nc.gpsimd.partition_broadcast(rb, rch)
                for hh in range(4):
                    h = 4 * gr + hh
                    nc.vector.tensor_mul(
                        attnT[ds(64 * (h % 2), 64), h // 2, ds(128 * c, P)],
                        oev[gr][:, ts(hh, P)],
                        rb[:, ts(hh, P)],
                    )

        emit_oproj(3, 1)

    nc.finalize()
    return nc


def _prep_inputs(x, Wq, Wkv, Wo):
    bf = ml_dtypes.bfloat16
    x = np.asarray(x, dtype=np.float32)
    Wq = np.asarray(Wq, dtype=np.float32)
    Wkv = np.asarray(Wkv, dtype=np.float32)
    Wo = np.asarray(Wo, dtype=np.float32)

    xT = [np.ascontiguousarray(x[b].T).astype(bf) for b in range(2)]
    wq = np.ascontiguousarray(Wq * SCALE).astype(bf)
    wkv = np.ascontiguousarray(Wkv).astype(bf)
    wo = np.ascontiguousarray(Wo).astype(bf)

    # triangular within-tile causal mask, replicated across the 8 head slots
    kk = np.arange(P)
    tri = (kk[:, None] <= kk[None, :]).astype(bf)          # [128 k, 128 q]
    tri8 = np.tile(tri, (1, 8))                            # [128, 1024]
    ones8 = np.ones((P, 1024), dtype=bf)

    in_maps = []
    for core in range(8):
        b, g = core // 4, core % 4
        qtiles = [4 * c + g for c in range(4)]
        xq = np.ascontiguousarray(
            np.concatenate([x[b, 128 * t : 128 * t + 128, :] for t in qtiles]).T
        ).astype(bf)
        maskq = np.concatenate(
            [tri8 if jd == g else ones8 for jd in range(4)], axis=1
        )  # [128, 4*1024]
        biasb = np.zeros((P, 4), dtype=np.float32)
        for jd in range(4):
            if jd > g:
                biasb[:, jd] = NEG
        in_maps.append(
            {
                "xT": xT[b],
                "xq": xq,
                "wq": wq,
                "wkv": wkv,
                "wo": wo,
                "maskq": np.ascontiguousarray(maskq),
                "biasb": biasb,
            }
        )
    return in_maps


def run(x, Wq, Wkv, Wo, trace=False, **trace_kwargs):
    global _PROGRAM
    if _PROGRAM is None:
        _PROGRAM = build_program()
    nc = _PROGRAM
    in_maps = _prep_inputs(x, Wq, Wkv, Wo)
    res = run_bass_kernel_spmd(
        nc, in_maps, core_ids=list(range(8)), trace=trace, **trace_kwargs
    )
    outs = res.results
    full = np.empty((2, T, HID), dtype=np.float32)
    for core in range(8):
        b, g = core // 4, core % 4
        outT = np.asarray(outs[core]["outT"]).astype(np.float32)
        for c in range(4):
            t = 4 * c + g
            full[b, 128 * t : 128 * t + 128, :] = outT[:, 128 * c : 128 * c + 128].T
    return full, res


def kernel(x, Wq, Wkv, Wo):
    out, _ = run(x, Wq, Wkv, Wo, trace=False)
    return out
